# revision 5
# baseline (speedup 1.0000x reference)
"""CharCNN embedder (ELMo-style) Trainium2 Bass kernel.

Strategy (pure data parallel over 8 cores, 256 tokens each):
  - Embedding lookup as one-hot matmul: ids replicated across partitions via a
    K=1 ones-matmul, one-hot built with is_equal compares (DVE/Pool), then
    embT.T @ onehot accumulated over 3 row-chunks of the 262-entry table
    -> xT [16, 12800] directly in conv layout (no transposes, no gathers).
  - im2col by 6 shifted SBUF->SBUF DMA copies -> X7 [112+6, 12800+pad].
    Rows 112..117 are per-position-class indicator rows; the conv weight
    matrix carries -1e30 in those rows for (channel, position) pairs that are
    invalid, so max-over-time needs no masking pass.
  - All 7 convs as one packed [118, 2048] bf16 matmul; max-over-time as a
    strided reduce straight out of PSUM, split across DVE / ACT+DVE / ACT+Pool.
  - Highway + projection in channel-major form (hT [2048, 256]) with weight
    slabs streamed from DRAM in bf16.
"""

import os
import numpy as np
import ml_dtypes

import concourse.bass as bass
import concourse.mybir as mybir
import concourse.tile as tile
from concourse.bass_utils import run_bass_kernel_spmd

F32 = mybir.dt.float32
BF16 = mybir.dt.bfloat16
FP16 = mybir.dt.float16
NPBF16 = ml_dtypes.bfloat16

CNN_OPTIONS = [(1, 32), (2, 32), (3, 64), (4, 128), (5, 256), (6, 512), (7, 1024)]
EMB_DIM = 16
N_CHARS = 262
MAX_CHARS = 50
N_FILTERS = 2048
OUT_DIM = 512
N_HIGHWAY = 2
BATCH, SEQ = 4, 512
NCORES = 8
T_LOC = BATCH * SEQ // NCORES          # 256 tokens per core
COLS = T_LOC * MAX_CHARS               # 12800
COLS_PAD = COLS + 16                   # 12816
KMAX = 7
KROWS = EMB_DIM * KMAX                 # 112
KTOT = KROWS + 6                       # 118 (6 indicator rows for pos 44..49)
NCH = 512                              # xT build chunk width
NXCH = COLS // NCH                     # 25
TOKG = 8                               # tokens per conv chunk
NTG = T_LOC // TOKG                    # 32 conv N-chunks
# oc-chunk list: (chunk idx -> kernel size driving its valid-position count)
# channel order = concat order: k1(32) k2(32) k3(64) | k4(128) | k5(256) | k6(512)x? ...
CHUNK_K = [1, 4, 5, 5, 6, 6, 6, 6, 7, 7, 7, 7, 7, 7, 7, 7]  # m=0 mixed (use 50 pos)
CHUNK_NP = [50 if k == 1 else (MAX_CHARS - k + 1) for k in CHUNK_K]
HWM = 32                               # 4096/128 output chunks per highway layer
KC = 16                                # 2048/128 contraction chunks


def _split_multi_waits(nc):
    """This walrus build encodes at most ONE sync-wait per instruction.
    Hoist extra waits onto dedicated NoOps ahead of the instruction."""
    ctr = [0]
    for f in nc.m.functions:
        for b in f.blocks:
            il = b.instructions
            if not any(
                i.sync_info is not None and len(i.sync_info.on_wait) > 1 for i in il
            ):
                continue
            new = []
            for ins in il:
                si = ins.sync_info
                if si is not None and len(si.on_wait) > 1:
                    waits = list(si.on_wait)
                    for w in waits[:-1]:
                        ctr[0] += 1
                        nop = mybir.InstNoOp(name=f"wsplit-{ctr[0]}", ins=[], outs=[])
                        nop.engine = ins.engine
                        nop.sync_info = mybir.SyncInfo(on_wait=[w], on_update=[])
                        new.append(nop)
                    ins.sync_info = mybir.SyncInfo(
                        on_wait=[waits[-1]], on_update=list(si.on_update)
                    )
                new.append(ins)
            b.instructions = new


def _build_program():
    nc = bass.Bass(target_bir_lowering=False)

    ids_d = nc.dram_tensor("ids", [1, COLS_PAD], FP16, kind="ExternalInput")
    iota_d = nc.dram_tensor("iota3", [128, 4], F32, kind="ExternalInput")
    embt_d = nc.dram_tensor("embt", [384, EMB_DIM], BF16, kind="ExternalInput")
    convw_d = nc.dram_tensor("convw", [KTOT, N_FILTERS], BF16, kind="ExternalInput")
    indic_d = nc.dram_tensor("indic", [6, COLS_PAD], BF16, kind="ExternalInput")
    cbias_d = nc.dram_tensor("cbias", [128, 16], F32, kind="ExternalInput")
    hw0_d = nc.dram_tensor("hw0", [HWM, 128, 2048], BF16, kind="ExternalInput")
    hw1_d = nc.dram_tensor("hw1", [HWM, 128, 2048], BF16, kind="ExternalInput")
    hb0_d = nc.dram_tensor("hb0", [128, HWM], F32, kind="ExternalInput")
    hb1_d = nc.dram_tensor("hb1", [128, HWM], F32, kind="ExternalInput")
    pw_d = nc.dram_tensor("pw", [4, 128, 2048], BF16, kind="ExternalInput")
    pb_d = nc.dram_tensor("pb", [128, 4], F32, kind="ExternalInput")
    out_d = nc.dram_tensor("outT", [OUT_DIM, T_LOC], F32, kind="ExternalOutput")

    with tile.TileContext(nc) as tc:
        with (
            tc.tile_pool(name="const", bufs=1) as cpool,
            tc.tile_pool(name="oh", bufs=3) as ohpool,
            tc.tile_pool(name="idsrep", bufs=3) as idpool,
            tc.tile_pool(name="stg", bufs=4) as stgpool,
            tc.tile_pool(name="wslab", bufs=3) as wpool,
            tc.tile_pool(name="elem", bufs=3) as epool,
            tc.tile_pool(name="outp", bufs=2) as outpool,
            tc.tile_pool(name="ps_big", bufs=3, space="PSUM") as ps_big,
            tc.tile_pool(name="ps_xt", bufs=1, space="PSUM") as ps_xt,
            tc.tile_pool(name="ps_hw", bufs=2, space="PSUM") as ps_hw,
        ):
            # ---- constants in ----
            ids_s = cpool.tile([1, COLS_PAD], FP16, tag="ids")
            nc.sync.dma_start(ids_s[:], ids_d[:])
            iota_s = cpool.tile([128, 4], F32, tag="iota")
            nc.sync.dma_start(iota_s[:], iota_d[:])
            embt_s = cpool.tile([128, 3 * EMB_DIM], BF16, tag="embt")
            for r in range(3):
                nc.sync.dma_start(
                    embt_s[:, 16 * r : 16 * r + 16], embt_d[128 * r : 128 * r + 128, :]
                )
            convw_s = cpool.tile([KTOT, N_FILTERS], BF16, tag="convw")
            nc.sync.dma_start(convw_s[:], convw_d[:])
            cbias_s = cpool.tile([128, 16], F32, tag="cbias")
            nc.sync.dma_start(cbias_s[:], cbias_d[:])
            hb0_s = cpool.tile([128, HWM], F32, tag="hb0")
            nc.sync.dma_start(hb0_s[:], hb0_d[:])
            hb1_s = cpool.tile([128, HWM], F32, tag="hb1")
            nc.sync.dma_start(hb1_s[:], hb1_d[:])
            pb_s = cpool.tile([128, 4], F32, tag="pb")
            nc.sync.dma_start(pb_s[:], pb_d[:])
            ones_s = cpool.tile([1, 128], FP16, tag="ones")
            nc.gpsimd.memset(ones_s[:], 1.0)

            # X7: rows 0-15 xT base, 16-111 shifted copies, 112-117 indicators
            X7 = cpool.tile([KTOT, COLS_PAD], BF16, tag="X7")
            nc.sync.dma_start(X7[112:118, :], indic_d[:])
            nc.gpsimd.memset(X7[0:16, COLS:COLS_PAD], 0.0)

            hts = [cpool.tile([128, T_LOC], BF16, tag=f"hT{m}", name=f"hT{m}") for m in range(KC)]
            h1ts = [cpool.tile([128, T_LOC], BF16, tag=f"h1T{m}", name=f"h1T{m}") for m in range(KC)]
            h2ts = [cpool.tile([128, T_LOC], BF16, tag=f"h2T{m}", name=f"h2T{m}") for m in range(KC)]

            # ---- phase B: xT via one-hot matmul ----
            for n in range(NXCH):
                c0 = n * NCH
                ps_ids = ps_big.tile([128, NCH], F32, space="PSUM", tag="big")
                nc.tensor.matmul(
                    ps_ids[:],
                    ones_s[0:1, :],
                    ids_s[0:1, c0 : c0 + NCH],
                    start=True,
                    stop=True,
                )
                idr = idpool.tile([128, NCH], FP16, tag="idr")
                nc.scalar.copy(idr[:], ps_ids[:])
                ohs = []
                for r in range(3):
                    oh = ohpool.tile([128, NCH], BF16, tag=f"oh{r}")
                    eng = nc.vector if r == 0 else nc.gpsimd
                    eng.tensor_scalar(
                        out=oh[:],
                        in0=idr[:],
                        scalar1=iota_s[:, r : r + 1],
                        scalar2=None,
                        op0=mybir.AluOpType.is_equal,
                    )
                    ohs.append(oh)
                px = ps_xt.tile([16, NCH], F32, space="PSUM", tag="xt")
                for r in range(3):
                    nc.tensor.matmul(
                        px[:],
                        embt_s[:, 16 * r : 16 * r + 16],
                        ohs[r][:],
                        start=(r == 0),
                        stop=(r == 2),
                    )
                nc.scalar.copy(X7[0:16, c0 : c0 + NCH], px[:])

            # ---- phase C: shifted im2col rows ----
            for j in range(1, KMAX):
                nc.sync.dma_start(
                    X7[16 * j : 16 * j + 16, 0 : COLS + 6],
                    X7[0:16, j : j + COLS + 6],
                )

            # ---- phase D: conv + max-over-time ----
            rr = 0
            for m in range(16):
                npos = CHUNK_NP[m]
                for n in range(NTG):
                    c0 = n * TOKG * MAX_CHARS
                    ps = ps_big.tile([128, TOKG * npos], F32, space="PSUM", tag="big")
                    rhs = (
                        X7[0:KTOT, c0 : c0 + TOKG * MAX_CHARS]
                        .rearrange("p (t c) -> p t c", c=MAX_CHARS)[:, :, 0:npos]
                    )
                    nc.tensor.matmul(
                        ps[:],
                        convw_s[:, 128 * m : 128 * m + 128],
                        rhs,
                        start=True,
                        stop=True,
                    )
                    dst = hts[m][:, TOKG * n : TOKG * n + TOKG]
                    path = rr % 4
                    rr += 1
                    if path == 0:
                        nc.vector.reduce_max(
                            dst,
                            ps[:].rearrange("p (t c) -> p t c", c=npos),
                            axis=mybir.AxisListType.X,
                        )
                    else:
                        stg = stgpool.tile([128, TOKG * npos], BF16, tag="stg")
                        nc.scalar.copy(stg[:], ps[:])
                        nc.vector.reduce_max(
                            dst,
                            stg[:].rearrange("p (t c) -> p t c", c=npos),
                            axis=mybir.AxisListType.X,
                        )
                nc.scalar.activation(
                    hts[m][:],
                    hts[m][:],
                    mybir.ActivationFunctionType.Relu,
                    bias=cbias_s[:, m : m + 1],
                    scale=1.0,
                )

            # ---- phase E: highway layers ----
            for lyr in range(N_HIGHWAY):
                src = hts if lyr == 0 else h1ts
                dst = h1ts if lyr == 0 else h2ts
                w_d = hw0_d if lyr == 0 else hw1_d
                hb_s = hb0_s if lyr == 0 else hb1_s
                for c in range(KC):
                    pss = {}
                    for kind, m in (("n", c), ("g", KC + c)):
                        slab = wpool.tile([128, 2048], BF16, tag="wslab")
                        nc.sync.dma_start(slab[:], w_d[m])
                        ps = ps_hw.tile(
                            [128, T_LOC], F32, space="PSUM", tag=f"hw{kind}"
                        )
                        for k in range(KC):
                            nc.tensor.matmul(
                                ps[:],
                                slab[:, 128 * k : 128 * k + 128],
                                src[k][:],
                                start=(k == 0),
                                stop=(k == KC - 1),
                            )
                        pss[kind] = ps
                    r_ = epool.tile([128, T_LOC], BF16, tag="relu")
                    nc.scalar.activation(
                        r_[:],
                        pss["n"][:],
                        mybir.ActivationFunctionType.Relu,
                        bias=hb_s[:, c : c + 1],
                        scale=1.0,
                    )
                    g_ = epool.tile([128, T_LOC], BF16, tag="gate")
                    nc.scalar.activation(
                        g_[:],
                        pss["g"][:],
                        mybir.ActivationFunctionType.Sigmoid,
                        bias=hb_s[:, KC + c : KC + c + 1],
                        scale=1.0,
                    )
                    t1 = epool.tile([128, T_LOC], BF16, tag="t1")
                    nc.vector.tensor_tensor(
                        out=t1[:], in0=src[c][:], in1=r_[:], op=mybir.AluOpType.subtract
                    )
                    t2 = epool.tile([128, T_LOC], BF16, tag="t2")
                    nc.vector.tensor_tensor(
                        out=t2[:], in0=g_[:], in1=t1[:], op=mybir.AluOpType.mult
                    )
                    nc.vector.tensor_tensor(
                        out=dst[c][:], in0=t2[:], in1=r_[:], op=mybir.AluOpType.add
                    )

            # ---- phase F: projection ----
            for m in range(4):
                slab = wpool.tile([128, 2048], BF16, tag="wslab")
                nc.sync.dma_start(slab[:], pw_d[m])
                ps = ps_hw.tile([128, T_LOC], F32, space="PSUM", tag="hwn")
                for k in range(KC):
                    nc.tensor.matmul(
                        ps[:],
                        slab[:, 128 * k : 128 * k + 128],
                        h2ts[k][:],
                        start=(k == 0),
                        stop=(k == KC - 1),
                    )
                oc = outpool.tile([128, T_LOC], F32, tag="out")
                nc.scalar.activation(
                    oc[:],
                    ps[:],
                    mybir.ActivationFunctionType.Identity,
                    bias=pb_s[:, m : m + 1],
                    scale=1.0,
                )
                nc.sync.dma_start(out_d[128 * m : 128 * m + 128, :], oc[:])

    _split_multi_waits(nc)
    return nc


def _prep_weights(inputs):
    conv_ws = [np.asarray(inputs[f"conv_w{i}"], np.float32) for i in range(7)]
    conv_bs = [np.asarray(inputs[f"conv_b{i}"], np.float32) for i in range(7)]

    W7 = np.zeros((KTOT, N_FILTERS), np.float32)
    o0 = 0
    for (ksz, oc), w in zip(CNN_OPTIONS, conv_ws):
        for j in range(ksz):
            W7[16 * j : 16 * j + 16, o0 : o0 + oc] = w[:, :, j].T
        # indicator-mask rows: position p = 44+i invalid iff p > 50-ksz
        for i in range(6):
            if (44 + i) > (MAX_CHARS - ksz):
                W7[KROWS + i, o0 : o0 + oc] = -1e30
        o0 += oc

    b_all = np.concatenate(conv_bs)
    cbias = b_all.reshape(16, 128).T.astype(np.float32)

    indic = np.zeros((6, COLS_PAD), np.float32)
    for i in range(6):
        indic[i, (44 + i) : COLS : MAX_CHARS] = 1.0

    emb = np.asarray(inputs["emb"], np.float32)
    embt = np.zeros((384, EMB_DIM), np.float32)
    embt[:N_CHARS] = emb

    iota = np.zeros((128, 4), np.float32)
    for r in range(3):
        iota[:, r] = np.arange(128) + 128 * r
    iota[:, 3] = 1000.0  # never matches

    def slab(w, mchunks):
        # S[m][kp, 128k+mc] = w[128m+mc, 128k+kp]
        a = np.asarray(w, np.float32).reshape(mchunks, 128, KC, 128)
        return np.ascontiguousarray(a.transpose(0, 3, 2, 1)).reshape(
            mchunks, 128, 2048
        )

    hw0 = slab(inputs["hw_w0"], HWM)
    hw1 = slab(inputs["hw_w1"], HWM)
    hb0 = np.asarray(inputs["hw_b0"], np.float32).reshape(HWM, 128).T.copy()
    hb1 = np.asarray(inputs["hw_b1"], np.float32).reshape(HWM, 128).T.copy()
    pw = slab(inputs["proj_w"], 4)
    pb = np.asarray(inputs["proj_b"], np.float32).reshape(4, 128).T.copy()

    return {
        "iota3": iota,
        "embt": embt.astype(NPBF16),
        "convw": W7.astype(NPBF16),
        "indic": indic.astype(NPBF16),
        "cbias": cbias,
        "hw0": hw0.astype(NPBF16),
        "hw1": hw1.astype(NPBF16),
        "hb0": hb0,
        "hb1": hb1,
        "pw": pw.astype(NPBF16),
        "pb": pb,
    }


_NC_CACHE = []
LAST_RESULT = {}


def kernel(**inputs) -> np.ndarray:
    if not _NC_CACHE:
        _NC_CACHE.append(_build_program())
    nc = _NC_CACHE[0]

    shared = _prep_weights(inputs)
    ids = np.asarray(inputs["batch_ids"]).astype(np.int64).reshape(-1, MAX_CHARS)
    in_maps = []
    for core in range(NCORES):
        flat = ids[core * T_LOC : (core + 1) * T_LOC].reshape(-1)
        idsp = np.zeros((1, COLS_PAD), np.float16)
        idsp[0, :COLS] = flat.astype(np.float16)
        in_maps.append({"ids": idsp, **shared})

    trace = bool(int(os.environ.get("KERNEL_TRACE", "0")))
    res = run_bass_kernel_spmd(
        nc, in_maps, core_ids=list(range(NCORES)), trace=trace
    )
    LAST_RESULT["exec_time_ns"] = res.exec_time_ns
    LAST_RESULT["trace"] = res.instructions_and_trace

    parts = [res.results[c]["outT"].T for c in range(NCORES)]  # each [256, 512]
    out = np.concatenate(parts, axis=0).reshape(BATCH, SEQ, OUT_DIM)
    return np.ascontiguousarray(out.astype(np.float32))


# revision 10
# speedup vs baseline: 1.3832x; 1.3832x over previous
"""CharCNN embedder (ELMo-style) Trainium2 Bass kernel.

Strategy (pure data parallel over 8 cores, 256 tokens each):
  - Embedding lookup as one-hot matmul: ids replicated across partitions via a
    K=1 ones-matmul, one-hot built with is_equal compares (DVE/Pool), then
    embT.T @ onehot accumulated over 3 row-chunks of the 262-entry table
    -> xT [16, 12800] directly in conv layout (no transposes, no gathers).
  - im2col by 6 shifted SBUF->SBUF DMA copies -> X7 [112+6, 12800+pad].
    Rows 112..117 are per-position-class indicator rows; the conv weight
    matrix carries -1e30 in those rows for (channel, position) pairs that are
    invalid, so max-over-time needs no masking pass.
  - All 7 convs as one packed [118, 2048] bf16 matmul; max-over-time as a
    strided reduce straight out of PSUM, split across DVE / ACT+DVE / ACT+Pool.
  - Highway + projection in channel-major form (hT [2048, 256]) with weight
    slabs streamed from DRAM in bf16.
"""

import os
import numpy as np
import ml_dtypes

import concourse.bass as bass
import concourse.mybir as mybir
import concourse.tile as tile
from concourse.bass_utils import run_bass_kernel_spmd

F32 = mybir.dt.float32
BF16 = mybir.dt.bfloat16
FP16 = mybir.dt.float16
NPBF16 = ml_dtypes.bfloat16

CNN_OPTIONS = [(1, 32), (2, 32), (3, 64), (4, 128), (5, 256), (6, 512), (7, 1024)]
EMB_DIM = 16
N_CHARS = 262
MAX_CHARS = 50
N_FILTERS = 2048
OUT_DIM = 512
N_HIGHWAY = 2
BATCH, SEQ = 4, 512
NCORES = 8
T_LOC = BATCH * SEQ // NCORES          # 256 tokens per core
COLS = T_LOC * MAX_CHARS               # 12800
COLS_PAD = COLS + 16                   # 12816
KMAX = 7
KROWS = EMB_DIM * KMAX                 # 112
KTOT = KROWS + 6                       # 118 (6 indicator rows for pos 44..49)
NCH = 512                              # xT build chunk width
NXCH = COLS // NCH                     # 25
TOKG = 8                               # tokens per conv chunk
NTG = T_LOC // TOKG                    # 32 conv N-chunks
# oc-chunk list: (chunk idx -> kernel size driving its valid-position count)
# channel order = concat order: k1(32) k2(32) k3(64) | k4(128) | k5(256) | k6(512)x? ...
CHUNK_K = [1, 4, 5, 5, 6, 6, 6, 6, 7, 7, 7, 7, 7, 7, 7, 7]  # m=0 mixed (use 50 pos)
CHUNK_NP = [50 if k == 1 else (MAX_CHARS - k + 1) for k in CHUNK_K]
HWM = 32                               # 4096/128 output chunks per highway layer
KC = 16                                # 2048/128 contraction chunks


def _split_multi_waits(nc):
    """This walrus build encodes at most ONE sync-wait per instruction.
    Hoist extra waits onto dedicated NoOps ahead of the instruction."""
    ctr = [0]
    for f in nc.m.functions:
        for b in f.blocks:
            il = b.instructions
            if not any(
                i.sync_info is not None and len(i.sync_info.on_wait) > 1 for i in il
            ):
                continue
            new = []
            for ins in il:
                si = ins.sync_info
                if si is not None and len(si.on_wait) > 1:
                    waits = list(si.on_wait)
                    for w in waits[:-1]:
                        ctr[0] += 1
                        nop = mybir.InstNoOp(name=f"wsplit-{ctr[0]}", ins=[], outs=[])
                        nop.engine = ins.engine
                        nop.sync_info = mybir.SyncInfo(on_wait=[w], on_update=[])
                        new.append(nop)
                    ins.sync_info = mybir.SyncInfo(
                        on_wait=[waits[-1]], on_update=list(si.on_update)
                    )
                new.append(ins)
            b.instructions = new


def _build_program():
    nc = bass.Bass(target_bir_lowering=False)

    ids_d = nc.dram_tensor("ids", [1, COLS_PAD], FP16, kind="ExternalInput")
    iota_d = nc.dram_tensor("iota3", [128, 4], F32, kind="ExternalInput")
    embt_d = nc.dram_tensor("embt", [384, EMB_DIM], BF16, kind="ExternalInput")
    convw_d = nc.dram_tensor("convw", [KTOT, N_FILTERS], BF16, kind="ExternalInput")
    indic_d = nc.dram_tensor("indic", [6, COLS_PAD], BF16, kind="ExternalInput")
    cbias_d = nc.dram_tensor("cbias", [128, 16], F32, kind="ExternalInput")
    hw0_d = nc.dram_tensor("hw0", [KC, 8, 128, 512], BF16, kind="ExternalInput")
    hw1_d = nc.dram_tensor("hw1", [KC, 8, 128, 512], BF16, kind="ExternalInput")
    hb0_d = nc.dram_tensor("hb0", [1, 4096], BF16, kind="ExternalInput")
    hb1_d = nc.dram_tensor("hb1", [1, 4096], BF16, kind="ExternalInput")
    pw_d = nc.dram_tensor("pw", [KC, 128, 512], BF16, kind="ExternalInput")
    pb_d = nc.dram_tensor("pb", [1, 512], BF16, kind="ExternalInput")
    out_d = nc.dram_tensor("outT", [T_LOC, OUT_DIM], F32, kind="ExternalOutput")

    with tile.TileContext(nc) as tc:
        with (
            tc.tile_pool(name="const", bufs=1) as cpool,
            tc.tile_pool(name="oh", bufs=3) as ohpool,
            tc.tile_pool(name="idsrep", bufs=3) as idpool,
            tc.tile_pool(name="stg", bufs=4) as stgpool,
            tc.tile_pool(name="wslab", bufs=20) as wpool,
            tc.tile_pool(name="elem", bufs=3) as epool,
            tc.tile_pool(name="outp", bufs=2) as outpool,
            tc.tile_pool(name="ps_big", bufs=2, space="PSUM") as ps_big,
            tc.tile_pool(name="ps_xt", bufs=2, space="PSUM") as ps_xt,
            tc.tile_pool(name="ps_hw", bufs=4, space="PSUM") as ps_hw,
        ):
            # ---- constants in ----
            ids_s = cpool.tile([1, COLS_PAD], FP16, tag="ids")
            nc.sync.dma_start(ids_s[:], ids_d[:])
            iota_s = cpool.tile([128, 4], F32, tag="iota")
            nc.sync.dma_start(iota_s[:], iota_d[:])
            embt_s = cpool.tile([128, 3 * EMB_DIM], BF16, tag="embt")
            for r in range(3):
                nc.sync.dma_start(
                    embt_s[:, 16 * r : 16 * r + 16], embt_d[128 * r : 128 * r + 128, :]
                )
            convw_s = cpool.tile([KTOT, N_FILTERS], BF16, tag="convw")
            nc.sync.dma_start(convw_s[:], convw_d[:])
            cbias_s = cpool.tile([128, 16], F32, tag="cbias")
            nc.sync.dma_start(cbias_s[:], cbias_d[:])
            hb0_s = cpool.tile([1, 4096], BF16, tag="hb0")
            nc.sync.dma_start(hb0_s[:], hb0_d[:])
            hb1_s = cpool.tile([1, 4096], BF16, tag="hb1")
            nc.sync.dma_start(hb1_s[:], hb1_d[:])
            pb_s = cpool.tile([1, 512], BF16, tag="pb")
            nc.sync.dma_start(pb_s[:], pb_d[:])
            ones_s = cpool.tile([1, 128], FP16, tag="ones")
            nc.gpsimd.memset(ones_s[:], 1.0)
            onesb_s = cpool.tile([1, 128], BF16, tag="onesb")
            nc.gpsimd.memset(onesb_s[:], 1.0)
            from concourse.masks import make_identity
            ident_s = cpool.tile([128, 128], BF16, tag="ident")
            make_identity(nc, ident_s[:])

            # X7: rows 0-15 xT base, 16-111 shifted copies, 112-117 indicators
            X7 = cpool.tile([KTOT, COLS_PAD], BF16, tag="X7")
            nc.sync.dma_start(X7[112:118, :], indic_d[:])
            nc.gpsimd.memset(X7[0:16, COLS:COLS_PAD], 0.0)

            hts = [cpool.tile([128, T_LOC], BF16, tag=f"hT{m}", name=f"hT{m}") for m in range(KC)]
            h1ts = [cpool.tile([128, T_LOC], BF16, tag=f"h1T{m}", name=f"h1T{m}") for m in range(KC)]
            h2ts = [cpool.tile([128, T_LOC], BF16, tag=f"h2T{m}", name=f"h2T{m}") for m in range(KC)]
            htok = [cpool.tile([128, N_FILTERS], BF16, tag=f"htok{t}", name=f"htok{t}") for t in range(2)]
            h1tok = [cpool.tile([128, N_FILTERS], BF16, tag=f"h1tok{t}", name=f"h1tok{t}") for t in range(2)]

            # ---- phase B: xT via one-hot matmul ----
            for n in range(NXCH):
                c0 = n * NCH
                ps_ids = ps_big.tile([128, NCH], F32, space="PSUM", tag="big")
                nc.tensor.matmul(
                    ps_ids[:],
                    ones_s[0:1, :],
                    ids_s[0:1, c0 : c0 + NCH],
                    start=True,
                    stop=True,
                )
                idr = idpool.tile([128, NCH], FP16, tag="idr")
                nc.scalar.copy(idr[:], ps_ids[:])
                ohs = []
                for r in range(3):
                    oh = ohpool.tile([128, NCH], BF16, tag=f"oh{r}")
                    nc.vector.tensor_scalar(
                        out=oh[:],
                        in0=idr[:],
                        scalar1=iota_s[:, r : r + 1],
                        scalar2=None,
                        op0=mybir.AluOpType.is_equal,
                    )
                    ohs.append(oh)
                px = ps_xt.tile([16, NCH], F32, space="PSUM", tag="xt")
                for r in range(3):
                    nc.tensor.matmul(
                        px[:],
                        embt_s[:, 16 * r : 16 * r + 16],
                        ohs[r][:],
                        start=(r == 0),
                        stop=(r == 2),
                    )
                nc.scalar.copy(X7[0:16, c0 : c0 + NCH], px[:])

            # ---- phase C: shifted im2col rows ----
            for j in range(1, KMAX):
                nc.sync.dma_start(
                    X7[16 * j : 16 * j + 16, 0 : COLS + 6],
                    X7[0:16, j : j + COLS + 6],
                )

            # ---- phase D: conv + max-over-time ----
            rr = 0
            for m in range(16):
                npos = CHUNK_NP[m]
                for n in range(NTG):
                    c0 = n * TOKG * MAX_CHARS
                    ps = ps_big.tile([128, TOKG * npos], F32, space="PSUM", tag="big")
                    rhs = (
                        X7[0:KTOT, c0 : c0 + TOKG * MAX_CHARS]
                        .rearrange("p (t c) -> p t c", c=MAX_CHARS)[:, :, 0:npos]
                    )
                    nc.tensor.matmul(
                        ps[:],
                        convw_s[:, 128 * m : 128 * m + 128],
                        rhs,
                        start=True,
                        stop=True,
                    )
                    dst = hts[m][:, TOKG * n : TOKG * n + TOKG]
                    path = rr % 2
                    rr += 1
                    if path == 0:
                        nc.vector.reduce_max(
                            dst,
                            ps[:].rearrange("p (t c) -> p t c", c=npos),
                            axis=mybir.AxisListType.X,
                        )
                    else:
                        stg = stgpool.tile([128, TOKG * npos], BF16, tag="stg")
                        nc.scalar.copy(stg[:], ps[:])
                        nc.vector.reduce_max(
                            dst,
                            stg[:].rearrange("p (t c) -> p t c", c=npos),
                            axis=mybir.AxisListType.X,
                        )
                nc.scalar.activation(
                    hts[m][:],
                    hts[m][:],
                    mybir.ActivationFunctionType.Relu,
                    bias=cbias_s[:, m : m + 1],
                    scale=1.0,
                )

            # ---- transpose helper: channel-major [128ch, 128tok] -> token-major ----
            def transpose_block(src_ap, dst_ap):
                tp = ps_xt.tile([128, 128], BF16, space="PSUM", tag="xt", name="tp")
                nc.tensor.transpose(tp[:], src_ap, ident_s[:])
                nc.vector.tensor_copy(dst_ap, tp[:])

            # hts (channel-major) -> htok (token-major)
            for mt in range(2):
                for c in range(KC):
                    transpose_block(
                        hts[c][:, 128 * mt : 128 * mt + 128],
                        htok[mt][:, 128 * c : 128 * c + 128],
                    )

            # ---- phases E/F: highway (token-major out) + projection ----
            # weight columns host-interleaved: slab b = [256 nonlin | 256 gate]
            # for channel block b, so each psum chunk is elementwise-complete.
            for lyr in range(N_HIGHWAY):
                src_ts = hts if lyr == 0 else h1ts
                h_in = htok if lyr == 0 else h1tok
                w_d = hw0_d if lyr == 0 else hw1_d
                hb_s = hb0_s if lyr == 0 else hb1_s
                for b in range(8):
                    slabs = []
                    for k in range(KC):
                        slab = wpool.tile(
                            [128, 512], BF16, tag="wslab", name=f"ws{lyr}{b}{k}"
                        )
                        nc.sync.dma_start(slab[:], w_d[k, b])
                        slabs.append(slab)
                    for mt in range(2):
                        ps = ps_hw.tile(
                            [128, 512], F32, space="PSUM", tag="hwp",
                            name=f"hwp{lyr}{b}{mt}",
                        )
                        for k in range(KC):
                            nc.tensor.matmul(
                                ps[:],
                                src_ts[k][:, 128 * mt : 128 * mt + 128],
                                slabs[k][:],
                                start=(k == 0),
                                stop=False,
                            )
                        nc.tensor.matmul(
                            ps[:],
                            onesb_s[0:1, :],
                            hb_s[0:1, 512 * b : 512 * b + 512],
                            start=False,
                            stop=True,
                        )
                        r_ = epool.tile([128, 256], BF16, tag="relu", name=f"r{lyr}{b}{mt}")
                        nc.scalar.activation(
                            r_[:], ps[:, 0:256], mybir.ActivationFunctionType.Relu
                        )
                        g_ = epool.tile([128, 256], BF16, tag="gate", name=f"g{lyr}{b}{mt}")
                        nc.scalar.activation(
                            g_[:], ps[:, 256:512], mybir.ActivationFunctionType.Sigmoid
                        )
                        hsl = h_in[mt][:, 256 * b : 256 * b + 256]
                        t1 = epool.tile([128, 256], BF16, tag="t1", name=f"t1_{lyr}{b}{mt}")
                        nc.vector.tensor_tensor(
                            out=t1[:], in0=hsl, in1=r_[:], op=mybir.AluOpType.subtract
                        )
                        t2 = epool.tile([128, 256], BF16, tag="t2", name=f"t2_{lyr}{b}{mt}")
                        nc.vector.tensor_tensor(
                            out=t2[:], in0=g_[:], in1=t1[:], op=mybir.AluOpType.mult
                        )
                        if lyr == 0:
                            dst_sl = h1tok[mt][:, 256 * b : 256 * b + 256]
                            nc.vector.tensor_tensor(
                                out=dst_sl, in0=t2[:], in1=r_[:], op=mybir.AluOpType.add
                            )
                            for c in (2 * b, 2 * b + 1):
                                transpose_block(
                                    h1tok[mt][:, 128 * c : 128 * c + 128],
                                    h1ts[c][:, 128 * mt : 128 * mt + 128],
                                )
                        else:
                            h2sl = epool.tile(
                                [128, 256], BF16, tag="h2sl", name=f"h2sl{b}{mt}"
                            )
                            nc.vector.tensor_tensor(
                                out=h2sl[:], in0=t2[:], in1=r_[:], op=mybir.AluOpType.add
                            )
                            for ci, c in enumerate((2 * b, 2 * b + 1)):
                                transpose_block(
                                    h2sl[:, 128 * ci : 128 * ci + 128],
                                    h2ts[c][:, 128 * mt : 128 * mt + 128],
                                )

            # ---- projection (token-major out) ----
            pslabs = []
            for k in range(KC):
                slab = wpool.tile([128, 512], BF16, tag="wslab", name=f"pws{k}")
                nc.sync.dma_start(slab[:], pw_d[k])
                pslabs.append(slab)
            for mt in range(2):
                ps = ps_hw.tile([128, 512], F32, space="PSUM", tag="hwp", name=f"pj{mt}")
                for k in range(KC):
                    nc.tensor.matmul(
                        ps[:],
                        h2ts[k][:, 128 * mt : 128 * mt + 128],
                        pslabs[k][:],
                        start=(k == 0),
                        stop=False,
                    )
                nc.tensor.matmul(
                    ps[:], onesb_s[0:1, :], pb_s[0:1, :], start=False, stop=True
                )
                oc = outpool.tile([128, 512], F32, tag="out", name=f"oc{mt}")
                nc.scalar.copy(oc[:], ps[:])
                nc.sync.dma_start(out_d[128 * mt : 128 * mt + 128, :], oc[:])

    _split_multi_waits(nc)
    return nc


def _prep_weights(inputs):
    conv_ws = [np.asarray(inputs[f"conv_w{i}"], np.float32) for i in range(7)]
    conv_bs = [np.asarray(inputs[f"conv_b{i}"], np.float32) for i in range(7)]

    W7 = np.zeros((KTOT, N_FILTERS), np.float32)
    o0 = 0
    for (ksz, oc), w in zip(CNN_OPTIONS, conv_ws):
        for j in range(ksz):
            W7[16 * j : 16 * j + 16, o0 : o0 + oc] = w[:, :, j].T
        # indicator-mask rows: position p = 44+i invalid iff p > 50-ksz
        for i in range(6):
            if (44 + i) > (MAX_CHARS - ksz):
                W7[KROWS + i, o0 : o0 + oc] = -1e30
        o0 += oc

    b_all = np.concatenate(conv_bs)
    cbias = b_all.reshape(16, 128).T.astype(np.float32)

    indic = np.zeros((6, COLS_PAD), np.float32)
    for i in range(6):
        indic[i, (44 + i) : COLS : MAX_CHARS] = 1.0

    emb = np.asarray(inputs["emb"], np.float32)
    embt = np.zeros((384, EMB_DIM), np.float32)
    embt[:N_CHARS] = emb

    iota = np.zeros((128, 4), np.float32)
    for r in range(3):
        iota[:, r] = np.arange(128) + 128 * r
    iota[:, 3] = 1000.0  # never matches

    # token-form highway: out = h @ w'.T computed as hT-chunks.T @ w'.T-slabs.
    # Channel-interleave so slab b holds [256 nonlin | 256 gate] of block b.
    perm = np.concatenate(
        [
            np.concatenate([np.arange(256 * b, 256 * b + 256),
                            2048 + np.arange(256 * b, 256 * b + 256)])
            for b in range(8)
        ]
    )

    def hw_slabs(w, bvec):
        wp = np.asarray(w, np.float32)[perm]       # [4096, 2048] permuted rows
        bp = np.asarray(bvec, np.float32)[perm]
        wt = wp.T                                   # [2048, 4096]
        arr = wt.reshape(KC, 128, 8, 512).transpose(0, 2, 1, 3)  # [16, 8, 128, 512]
        return np.ascontiguousarray(arr).astype(NPBF16), bp[None, :].astype(NPBF16)

    hw0, hb0 = hw_slabs(inputs["hw_w0"], inputs["hw_b0"])
    hw1, hb1 = hw_slabs(inputs["hw_w1"], inputs["hw_b1"])
    pwt = np.asarray(inputs["proj_w"], np.float32).T  # [2048, 512]
    pw = np.ascontiguousarray(pwt.reshape(KC, 128, 512)).astype(NPBF16)
    pb = np.asarray(inputs["proj_b"], np.float32)[None, :].astype(NPBF16)

    return {
        "iota3": iota,
        "embt": embt.astype(NPBF16),
        "convw": W7.astype(NPBF16),
        "indic": indic.astype(NPBF16),
        "cbias": cbias,
        "hw0": hw0.astype(NPBF16),
        "hw1": hw1.astype(NPBF16),
        "hb0": hb0,
        "hb1": hb1,
        "pw": pw.astype(NPBF16),
        "pb": pb,
    }


_NC_CACHE = []
LAST_RESULT = {}


def kernel(**inputs) -> np.ndarray:
    if not _NC_CACHE:
        _NC_CACHE.append(_build_program())
    nc = _NC_CACHE[0]

    shared = _prep_weights(inputs)
    ids = np.asarray(inputs["batch_ids"]).astype(np.int64).reshape(-1, MAX_CHARS)
    in_maps = []
    for core in range(NCORES):
        flat = ids[core * T_LOC : (core + 1) * T_LOC].reshape(-1)
        idsp = np.zeros((1, COLS_PAD), np.float16)
        idsp[0, :COLS] = flat.astype(np.float16)
        in_maps.append({"ids": idsp, **shared})

    trace = bool(int(os.environ.get("KERNEL_TRACE", "0")))
    res = run_bass_kernel_spmd(
        nc, in_maps, core_ids=list(range(NCORES)), trace=trace
    )
    LAST_RESULT["exec_time_ns"] = res.exec_time_ns
    LAST_RESULT["trace"] = res.instructions_and_trace

    parts = [res.results[c]["outT"] for c in range(NCORES)]  # each [256, 512]
    out = np.concatenate(parts, axis=0).reshape(BATCH, SEQ, OUT_DIM)
    return np.ascontiguousarray(out.astype(np.float32))


# revision 13
# speedup vs baseline: 1.7474x; 1.2633x over previous
"""CharCNN embedder (ELMo-style) Trainium2 Bass kernel.

Strategy (pure data parallel over 8 cores, 256 tokens each):
  - Embedding lookup as one-hot matmul: ids replicated across partitions via a
    K=1 ones-matmul, one-hot built with is_equal compares (DVE/Pool), then
    embT.T @ onehot accumulated over 3 row-chunks of the 262-entry table
    -> xT [16, 12800] directly in conv layout (no transposes, no gathers).
  - im2col by 6 shifted SBUF->SBUF DMA copies -> X7 [112+6, 12800+pad].
    Rows 112..117 are per-position-class indicator rows; the conv weight
    matrix carries -1e30 in those rows for (channel, position) pairs that are
    invalid, so max-over-time needs no masking pass.
  - All 7 convs as one packed [118, 2048] bf16 matmul; max-over-time as a
    strided reduce straight out of PSUM, split across DVE / ACT+DVE / ACT+Pool.
  - Highway + projection in channel-major form (hT [2048, 256]) with weight
    slabs streamed from DRAM in bf16.
"""

import os
import numpy as np
import ml_dtypes

import concourse.bass as bass
import concourse.mybir as mybir
import concourse.tile as tile
from concourse.bass_utils import run_bass_kernel_spmd

F32 = mybir.dt.float32
BF16 = mybir.dt.bfloat16
FP16 = mybir.dt.float16
NPBF16 = ml_dtypes.bfloat16

CNN_OPTIONS = [(1, 32), (2, 32), (3, 64), (4, 128), (5, 256), (6, 512), (7, 1024)]
EMB_DIM = 16
N_CHARS = 262
MAX_CHARS = 50
N_FILTERS = 2048
OUT_DIM = 512
N_HIGHWAY = 2
BATCH, SEQ = 4, 512
NCORES = 8
T_LOC = BATCH * SEQ // NCORES          # 256 tokens per core
COLS = T_LOC * MAX_CHARS               # 12800
COLS_PAD = COLS + 16                   # 12816
KMAX = 7
KROWS = EMB_DIM * KMAX                 # 112
KTOT = KROWS + 6                       # 118 (6 indicator rows for pos 44..49)
NCH = 512                              # xT build chunk width
NXCH = COLS // NCH                     # 25
TOKG = 8                               # tokens per conv chunk
NTG = T_LOC // TOKG                    # 32 conv N-chunks
# oc-chunk list: (chunk idx -> kernel size driving its valid-position count)
# channel order = concat order: k1(32) k2(32) k3(64) | k4(128) | k5(256) | k6(512)x? ...
CHUNK_K = [1, 4, 5, 5, 6, 6, 6, 6, 7, 7, 7, 7, 7, 7, 7, 7]  # m=0 mixed (use 50 pos)
CHUNK_NP = [50 if k == 1 else (MAX_CHARS - k + 1) for k in CHUNK_K]
HWM = 32                               # 4096/128 output chunks per highway layer
KC = 16                                # 2048/128 contraction chunks


def _split_multi_waits(nc):
    """This walrus build encodes at most ONE sync-wait per instruction.
    Hoist extra waits onto dedicated NoOps ahead of the instruction."""
    ctr = [0]
    for f in nc.m.functions:
        for b in f.blocks:
            il = b.instructions
            if not any(
                i.sync_info is not None and len(i.sync_info.on_wait) > 1 for i in il
            ):
                continue
            new = []
            for ins in il:
                si = ins.sync_info
                if si is not None and len(si.on_wait) > 1:
                    waits = list(si.on_wait)
                    for w in waits[:-1]:
                        ctr[0] += 1
                        nop = mybir.InstNoOp(name=f"wsplit-{ctr[0]}", ins=[], outs=[])
                        nop.engine = ins.engine
                        nop.sync_info = mybir.SyncInfo(on_wait=[w], on_update=[])
                        new.append(nop)
                    ins.sync_info = mybir.SyncInfo(
                        on_wait=[waits[-1]], on_update=list(si.on_update)
                    )
                new.append(ins)
            b.instructions = new


def _build_program():
    nc = bass.Bass(target_bir_lowering=False)

    ids_d = nc.dram_tensor("ids", [1, COLS_PAD], FP16, kind="ExternalInput")
    iota_d = nc.dram_tensor("iota3", [128, 4], F32, kind="ExternalInput")
    embt_d = nc.dram_tensor("embt", [384, EMB_DIM], BF16, kind="ExternalInput")
    convw_d = nc.dram_tensor("convw", [KTOT, N_FILTERS], BF16, kind="ExternalInput")
    indic_d = nc.dram_tensor("indic", [6, COLS_PAD], BF16, kind="ExternalInput")
    cbias_d = nc.dram_tensor("cbias", [128, 16], F32, kind="ExternalInput")
    hw0_d = nc.dram_tensor("hw0", [KC, 8, 128, 512], BF16, kind="ExternalInput")
    hw1_d = nc.dram_tensor("hw1", [KC, 8, 128, 512], BF16, kind="ExternalInput")
    hb0_d = nc.dram_tensor("hb0", [1, 4096], BF16, kind="ExternalInput")
    hb1_d = nc.dram_tensor("hb1", [1, 4096], BF16, kind="ExternalInput")
    pw_d = nc.dram_tensor("pw", [KC, 128, 512], BF16, kind="ExternalInput")
    pb_d = nc.dram_tensor("pb", [1, 512], BF16, kind="ExternalInput")
    out_d = nc.dram_tensor("outT", [T_LOC, OUT_DIM], F32, kind="ExternalOutput")

    with tile.TileContext(nc) as tc:
        with (
            tc.tile_pool(name="const", bufs=1) as cpool,
            tc.tile_pool(name="oh", bufs=3) as ohpool,
            tc.tile_pool(name="idsrep", bufs=3) as idpool,
            tc.tile_pool(name="stg", bufs=4) as stgpool,
            tc.tile_pool(name="wslab", bufs=20) as wpool,
            tc.tile_pool(name="elem", bufs=3) as epool,
            tc.tile_pool(name="outp", bufs=2) as outpool,
            tc.tile_pool(name="ps_big", bufs=3, space="PSUM") as ps_big,
            tc.tile_pool(name="ps_xt", bufs=2, space="PSUM") as ps_xt,
            tc.tile_pool(name="ps_hw", bufs=3, space="PSUM") as ps_hw,
        ):
            # ---- constants in ----
            ids_s = cpool.tile([1, COLS_PAD], FP16, tag="ids")
            nc.sync.dma_start(ids_s[:], ids_d[:])
            iota_s = cpool.tile([128, 4], F32, tag="iota")
            nc.sync.dma_start(iota_s[:], iota_d[:])
            embt_s = cpool.tile([128, 3 * EMB_DIM], BF16, tag="embt")
            for r in range(3):
                nc.sync.dma_start(
                    embt_s[:, 16 * r : 16 * r + 16], embt_d[128 * r : 128 * r + 128, :]
                )
            convw_s = cpool.tile([KTOT, N_FILTERS], BF16, tag="convw")
            nc.sync.dma_start(convw_s[:], convw_d[:])
            cbias_s = cpool.tile([128, 16], F32, tag="cbias")
            nc.sync.dma_start(cbias_s[:], cbias_d[:])
            hb0_s = cpool.tile([1, 4096], BF16, tag="hb0")
            nc.sync.dma_start(hb0_s[:], hb0_d[:])
            hb1_s = cpool.tile([1, 4096], BF16, tag="hb1")
            nc.sync.dma_start(hb1_s[:], hb1_d[:])
            pb_s = cpool.tile([1, 512], BF16, tag="pb")
            nc.sync.dma_start(pb_s[:], pb_d[:])
            ones_s = cpool.tile([1, 128], FP16, tag="ones")
            nc.gpsimd.memset(ones_s[:], 1.0)
            onesb_s = cpool.tile([1, 128], BF16, tag="onesb")
            nc.gpsimd.memset(onesb_s[:], 1.0)
            from concourse.masks import make_identity
            ident_s = cpool.tile([128, 128], BF16, tag="ident")
            make_identity(nc, ident_s[:])

            # X7: rows 0-15 xT base, 16-111 shifted copies, 112-117 indicators
            X7 = cpool.tile([KTOT, COLS_PAD], BF16, tag="X7")
            nc.sync.dma_start(X7[112:118, :], indic_d[:])
            nc.gpsimd.memset(X7[0:16, COLS:COLS_PAD], 0.0)

            hts = [cpool.tile([128, T_LOC], BF16, tag=f"hT{m}", name=f"hT{m}") for m in range(KC)]
            h1ts = [cpool.tile([128, T_LOC], BF16, tag=f"h1T{m}", name=f"h1T{m}") for m in range(KC)]
            h2ts = [cpool.tile([128, T_LOC], BF16, tag=f"h2T{m}", name=f"h2T{m}") for m in range(KC)]
            htok = [cpool.tile([128, N_FILTERS], BF16, tag=f"htok{t}", name=f"htok{t}") for t in range(2)]
            h1tok = [cpool.tile([128, N_FILTERS], BF16, tag=f"h1tok{t}", name=f"h1tok{t}") for t in range(2)]

            # ---- helpers ----
            def transpose_block(src_ap, dst_ap, nm):
                tp = ps_xt.tile([128, 128], BF16, space="PSUM", tag="xt", name=nm)
                nc.tensor.transpose(tp[:], src_ap, ident_s[:])
                nc.vector.tensor_copy(dst_ap, tp[:])

            def bphase(lo, hi):
                for n in range(lo, hi):
                    c0 = n * NCH
                    ps_ids = ps_big.tile(
                        [128, NCH], F32, space="PSUM", tag="big", name=f"pid{n}"
                    )
                    nc.tensor.matmul(
                        ps_ids[:], ones_s[0:1, :], ids_s[0:1, c0 : c0 + NCH],
                        start=True, stop=True,
                    )
                    idr = idpool.tile([128, NCH], FP16, tag="idr", name=f"idr{n}")
                    nc.scalar.copy(idr[:], ps_ids[:])
                    ohs = []
                    for r in range(3):
                        oh = ohpool.tile([128, NCH], BF16, tag=f"oh{r}", name=f"oh{r}_{n}")
                        nc.vector.tensor_scalar(
                            out=oh[:], in0=idr[:], scalar1=iota_s[:, r : r + 1],
                            scalar2=None, op0=mybir.AluOpType.is_equal,
                        )
                        ohs.append(oh)
                    px = ps_xt.tile([16, NCH], F32, space="PSUM", tag="xt", name=f"px{n}")
                    for r in range(3):
                        nc.tensor.matmul(
                            px[:], embt_s[:, 16 * r : 16 * r + 16], ohs[r][:],
                            start=(r == 0), stop=(r == 2),
                        )
                    nc.scalar.copy(X7[0:16, c0 : c0 + NCH], px[:])

            def shifts(cl, cu):
                for j in range(1, KMAX):
                    nc.sync.dma_start(
                        X7[16 * j : 16 * j + 16, cl:cu], X7[0:16, cl + j : cu + j]
                    )

            def conv_half(half):
                for nn in range(16 * half, 16 * half + 16):
                    c0 = nn * TOKG * MAX_CHARS
                    for m in range(16):
                        npos = CHUNK_NP[m]
                        ps = ps_big.tile(
                            [128, TOKG * npos], F32, space="PSUM", tag="big",
                            name=f"cv{m}_{nn}",
                        )
                        rhs = (
                            X7[0:KTOT, c0 : c0 + TOKG * MAX_CHARS]
                            .rearrange("p (t c) -> p t c", c=MAX_CHARS)[:, :, 0:npos]
                        )
                        nc.tensor.matmul(
                            ps[:], convw_s[:, 128 * m : 128 * m + 128], rhs,
                            start=True, stop=True,
                        )
                        nc.vector.reduce_max(
                            hts[m][:, TOKG * nn : TOKG * nn + TOKG],
                            ps[:].rearrange("p (t c) -> p t c", c=npos),
                            axis=mybir.AxisListType.X,
                        )
                for m in range(16):
                    hsl = hts[m][:, 128 * half : 128 * half + 128]
                    nc.scalar.activation(
                        hsl, hsl, mybir.ActivationFunctionType.Relu,
                        bias=cbias_s[:, m : m + 1], scale=1.0,
                    )
                for c in range(KC):
                    transpose_block(
                        hts[c][:, 128 * half : 128 * half + 128],
                        htok[half][:, 128 * c : 128 * c + 128],
                        f"tpc{half}_{c}",
                    )

            def hw_layer(lyr, mts):
                src_ts = hts if lyr == 0 else h1ts
                h_in = htok if lyr == 0 else h1tok
                w_d = hw0_d if lyr == 0 else hw1_d
                hb_s = hb0_s if lyr == 0 else hb1_s
                for b in range(8):
                    slabs = []
                    for k in range(KC):
                        slab = wpool.tile(
                            [128, 512], BF16, tag="wslab",
                            name=f"ws{lyr}{b}{k}_{mts[0]}",
                        )
                        nc.sync.dma_start(slab[:], w_d[k, b])
                        slabs.append(slab)
                    for mt in mts:
                        ps = ps_hw.tile(
                            [128, 512], F32, space="PSUM", tag="hwp",
                            name=f"hwp{lyr}{b}{mt}",
                        )
                        for k in range(KC):
                            nc.tensor.matmul(
                                ps[:],
                                src_ts[k][:, 128 * mt : 128 * mt + 128],
                                slabs[k][:],
                                start=(k == 0), stop=False,
                            )
                        nc.tensor.matmul(
                            ps[:], onesb_s[0:1, :],
                            hb_s[0:1, 512 * b : 512 * b + 512],
                            start=False, stop=True,
                        )
                        r_ = epool.tile([128, 256], BF16, tag="relu", name=f"r{lyr}{b}{mt}")
                        nc.scalar.activation(
                            r_[:], ps[:, 0:256], mybir.ActivationFunctionType.Relu
                        )
                        g_ = epool.tile([128, 256], BF16, tag="gate", name=f"g{lyr}{b}{mt}")
                        nc.scalar.activation(
                            g_[:], ps[:, 256:512], mybir.ActivationFunctionType.Sigmoid
                        )
                        hsl = h_in[mt][:, 256 * b : 256 * b + 256]
                        t1 = epool.tile([128, 256], BF16, tag="t1", name=f"t1_{lyr}{b}{mt}")
                        nc.vector.tensor_tensor(
                            out=t1[:], in0=hsl, in1=r_[:], op=mybir.AluOpType.subtract
                        )
                        t2 = epool.tile([128, 256], BF16, tag="t2", name=f"t2_{lyr}{b}{mt}")
                        nc.vector.tensor_tensor(
                            out=t2[:], in0=g_[:], in1=t1[:], op=mybir.AluOpType.mult
                        )
                        if lyr == 0:
                            dst_sl = h1tok[mt][:, 256 * b : 256 * b + 256]
                            nc.vector.tensor_tensor(
                                out=dst_sl, in0=t2[:], in1=r_[:], op=mybir.AluOpType.add
                            )
                            for c in (2 * b, 2 * b + 1):
                                transpose_block(
                                    h1tok[mt][:, 128 * c : 128 * c + 128],
                                    h1ts[c][:, 128 * mt : 128 * mt + 128],
                                    f"tp1_{b}{mt}{c}",
                                )
                        else:
                            h2sl = epool.tile(
                                [128, 256], BF16, tag="h2sl", name=f"h2sl{b}{mt}"
                            )
                            nc.vector.tensor_tensor(
                                out=h2sl[:], in0=t2[:], in1=r_[:], op=mybir.AluOpType.add
                            )
                            for ci, c in enumerate((2 * b, 2 * b + 1)):
                                transpose_block(
                                    h2sl[:, 128 * ci : 128 * ci + 128],
                                    h2ts[c][:, 128 * mt : 128 * mt + 128],
                                    f"tp2_{b}{mt}{c}",
                                )

            # ---- pipelined schedule ----
            bphase(0, 13)
            shifts(0, 6400)
            bphase(13, NXCH)
            shifts(6400, COLS + 6)
            conv_half(0)
            hw_layer(0, [0])
            conv_half(1)
            hw_layer(0, [1])
            hw_layer(1, [0, 1])

            # ---- projection (token-major out) ----
            pslabs = []
            for k in range(KC):
                slab = wpool.tile([128, 512], BF16, tag="wslab", name=f"pws{k}")
                nc.sync.dma_start(slab[:], pw_d[k])
                pslabs.append(slab)
            for mt in range(2):
                ps = ps_hw.tile([128, 512], F32, space="PSUM", tag="hwp", name=f"pj{mt}")
                for k in range(KC):
                    nc.tensor.matmul(
                        ps[:], h2ts[k][:, 128 * mt : 128 * mt + 128], pslabs[k][:],
                        start=(k == 0), stop=False,
                    )
                nc.tensor.matmul(
                    ps[:], onesb_s[0:1, :], pb_s[0:1, :], start=False, stop=True
                )
                oc = outpool.tile([128, 512], F32, tag="out", name=f"oc{mt}")
                nc.scalar.copy(oc[:], ps[:])
                nc.sync.dma_start(out_d[128 * mt : 128 * mt + 128, :], oc[:])

    _split_multi_waits(nc)
    return nc


def _prep_weights(inputs):
    conv_ws = [np.asarray(inputs[f"conv_w{i}"], np.float32) for i in range(7)]
    conv_bs = [np.asarray(inputs[f"conv_b{i}"], np.float32) for i in range(7)]

    W7 = np.zeros((KTOT, N_FILTERS), np.float32)
    o0 = 0
    for (ksz, oc), w in zip(CNN_OPTIONS, conv_ws):
        for j in range(ksz):
            W7[16 * j : 16 * j + 16, o0 : o0 + oc] = w[:, :, j].T
        # indicator-mask rows: position p = 44+i invalid iff p > 50-ksz
        for i in range(6):
            if (44 + i) > (MAX_CHARS - ksz):
                W7[KROWS + i, o0 : o0 + oc] = -1e30
        o0 += oc

    b_all = np.concatenate(conv_bs)
    cbias = b_all.reshape(16, 128).T.astype(np.float32)

    indic = np.zeros((6, COLS_PAD), np.float32)
    for i in range(6):
        indic[i, (44 + i) : COLS : MAX_CHARS] = 1.0

    emb = np.asarray(inputs["emb"], np.float32)
    embt = np.zeros((384, EMB_DIM), np.float32)
    embt[:N_CHARS] = emb

    iota = np.zeros((128, 4), np.float32)
    for r in range(3):
        iota[:, r] = np.arange(128) + 128 * r
    iota[:, 3] = 1000.0  # never matches

    # token-form highway: out = h @ w'.T computed as hT-chunks.T @ w'.T-slabs.
    # Channel-interleave so slab b holds [256 nonlin | 256 gate] of block b.
    perm = np.concatenate(
        [
            np.concatenate([np.arange(256 * b, 256 * b + 256),
                            2048 + np.arange(256 * b, 256 * b + 256)])
            for b in range(8)
        ]
    )

    def hw_slabs(w, bvec):
        wp = np.asarray(w, np.float32)[perm]       # [4096, 2048] permuted rows
        bp = np.asarray(bvec, np.float32)[perm]
        wt = wp.T                                   # [2048, 4096]
        arr = wt.reshape(KC, 128, 8, 512).transpose(0, 2, 1, 3)  # [16, 8, 128, 512]
        return np.ascontiguousarray(arr).astype(NPBF16), bp[None, :].astype(NPBF16)

    hw0, hb0 = hw_slabs(inputs["hw_w0"], inputs["hw_b0"])
    hw1, hb1 = hw_slabs(inputs["hw_w1"], inputs["hw_b1"])
    pwt = np.asarray(inputs["proj_w"], np.float32).T  # [2048, 512]
    pw = np.ascontiguousarray(pwt.reshape(KC, 128, 512)).astype(NPBF16)
    pb = np.asarray(inputs["proj_b"], np.float32)[None, :].astype(NPBF16)

    return {
        "iota3": iota,
        "embt": embt.astype(NPBF16),
        "convw": W7.astype(NPBF16),
        "indic": indic.astype(NPBF16),
        "cbias": cbias,
        "hw0": hw0.astype(NPBF16),
        "hw1": hw1.astype(NPBF16),
        "hb0": hb0,
        "hb1": hb1,
        "pw": pw.astype(NPBF16),
        "pb": pb,
    }


_NC_CACHE = []
LAST_RESULT = {}


def kernel(**inputs) -> np.ndarray:
    if not _NC_CACHE:
        _NC_CACHE.append(_build_program())
    nc = _NC_CACHE[0]

    shared = _prep_weights(inputs)
    ids = np.asarray(inputs["batch_ids"]).astype(np.int64).reshape(-1, MAX_CHARS)
    in_maps = []
    for core in range(NCORES):
        flat = ids[core * T_LOC : (core + 1) * T_LOC].reshape(-1)
        idsp = np.zeros((1, COLS_PAD), np.float16)
        idsp[0, :COLS] = flat.astype(np.float16)
        in_maps.append({"ids": idsp, **shared})

    trace = bool(int(os.environ.get("KERNEL_TRACE", "0")))
    res = run_bass_kernel_spmd(
        nc, in_maps, core_ids=list(range(NCORES)), trace=trace
    )
    LAST_RESULT["exec_time_ns"] = res.exec_time_ns
    LAST_RESULT["trace"] = res.instructions_and_trace

    parts = [res.results[c]["outT"] for c in range(NCORES)]  # each [256, 512]
    out = np.concatenate(parts, axis=0).reshape(BATCH, SEQ, OUT_DIM)
    return np.ascontiguousarray(out.astype(np.float32))
